# revision 36
# baseline (speedup 1.0000x reference)
"""MultiHeadSelfAttention with 3D RoPE + 3D relative position bias on 8 trn2 cores.

Sharding: tensor-parallel over heads (2 heads per core). Each core:
  - projects q/k/v for its 2 heads (plus rotate-half projections for RoPE),
  - computes scores^T = k~.q~ with the 3D relative bias folded into the
    matmul contraction via one-hot position encodings,
  - causal-masked unnormalized-exp softmax (scores bounded, no max needed),
  - PV matmul with an appended ones-column producing softmax denominators,
  - AllGather of per-head attention outputs, then a row-slice of the final
    output projection.
Host does only sharding/relayout of inputs and the final gather/transpose.
"""
import numpy as np

import concourse.bass as bass
import concourse.bacc as bacc
import concourse.tile as tile
import concourse.mybir as mybir
from concourse.bass_utils import run_bass_kernel_spmd
from concourse.masks import make_identity

B, S, D, H, HD = 1, 1863, 1024, 16, 64
NC = 8
HPC = H // NC  # heads per core = 2
BASE = 10000.0
MAX_DXY, MAX_DZ = 30, 8
PX = PY = 10
PZ = 12
SCALE = HD ** -0.5

F32 = mybir.dt.float32
F32R = mybir.dt.float32r
I32 = mybir.dt.int32

# q-chunks (PSUM bank = 512 fp32) and k-tiles (128 partitions)
CHUNKS = [(c * 512, min(512, S - c * 512)) for c in range((S + 511) // 512)]
KTILES = [(kt * 128, min(128, S - kt * 128)) for kt in range((S + 127) // 128)]
NKT = len(KTILES)  # 15
SP = S + 1  # f32r matmuls need even moving-dim N; pad rhs tensors to 1864
PERM = np.concatenate([np.arange(0, HD, 2), np.arange(1, HD, 2)])  # even, odd


def _build_nc(debug=False, stop_after=None, no_cc=False, ko=()):
    nc = bacc.Bacc("TRN2")

    xTd = nc.dram_tensor("xT", [D, S], F32R, kind="ExternalInput")
    wproj = nc.dram_tensor("wproj", [128, 8, 384], F32R, kind="ExternalInput")
    wout = nc.dram_tensor("wout", [128, 8, 128], F32R, kind="ExternalInput")
    pos3 = nc.dram_tensor("pos3", [3, S], I32, kind="ExternalInput")
    ropetab = nc.dram_tensor("ropetab", [96, 256], F32R, kind="ExternalInput")
    maug = nc.dram_tensor("maug", [96, HPC, 96], F32R, kind="ExternalInput")
    iota96 = nc.dram_tensor("iota96", [96, 1], F32, kind="ExternalInput")
    outT = nc.dram_tensor("outT", [128, S], F32, kind="ExternalOutput")
    dbg = {}
    if debug:
        for nm, shp in [("d_U", [96, S]), ("d_cosf", [128, S]),
                        ("d_sinf", [128, S]), ("d_qr", [128, S]),
                        ("d_kr", [128, S]), ("d_qaug", [96, HPC * S]),
                        ("d_vnat", [128, NKT * 130]),
                        ("d_call", [128, 8 * S])]:
            dbg[nm] = nc.dram_tensor(nm, shp, F32R, kind="ExternalOutput")
        for nm, shp in [("d_a0", [65, S]), ("d_a1", [65, S]),
                        ("d_agin", [128, S])]:
            dbg[nm] = nc.dram_tensor(nm, shp, F32, kind="ExternalOutput")

    with tile.TileContext(nc) as tc:
        with (
            tc.tile_pool(name="persist", bufs=1) as pp,
            tc.tile_pool(name="dram", bufs=1, space="DRAM") as dp,
        ):
            # ---- persistent tiles (live across phases) ----
            U = pp.tile([96, SP], F32R, name="U")
            qr = pp.tile([128, SP], F32R, name="qr")
            kr = pp.tile([128, SP], F32R, name="kr")
            qaug = pp.tile([96, HPC, SP], F32R, name="qaug")
            v_nat = pp.tile([128, NKT, 130], F32R, name="v_nat")
            ident = pp.tile([128, 128], F32R, name="ident")
            iot = pp.tile([96, 1], F32, name="iot")
            agin_s = pp.tile([128, S], F32R, name="agin_s")

            nc.sync.dma_start(iot[:], iota96[:])
            zcolF = pp.tile([128, 8], F32, name="zcolF")
            nc.gpsimd.memset(zcolF[:], 0.0)
            nc.vector.tensor_copy(U[:, S:SP], zcolF[0:96, 0:1])
            nc.vector.tensor_copy(qr[:, S:SP], zcolF[:, 0:1])
            nc.vector.tensor_copy(kr[:, S:SP], zcolF[:, 0:1])
            nc.vector.tensor_copy(
                qaug[:, :, S:SP].rearrange("p h x -> p (h x)"),
                zcolF[0:96, 0:2])
            attnh = [pp.tile([65, S], F32, name=f"attnh{h}")
                     for h in range(HPC)]
            zrecs = [pp.tile([1, S], F32, name=f"zrec{h}")
                     for h in range(HPC)]
            zbs = [pp.tile([64, S], F32, name=f"zb{h}") for h in range(HPC)]
            identF = pp.tile([128, 128], F32, name="identF")
            make_identity(nc, identF[:])
            nc.vector.tensor_copy(ident[:], identF[:])
            # ones columns of v_nat (cols 64 and 129 of each k-tile block)
            onesF = pp.tile([128, NKT, 2], F32, name="onesF")
            nc.gpsimd.memset(onesF[:], 1.0)
            ones_ap = v_nat[:].rearrange("p k (h x) -> p k h x", h=2)[:, :, :, 64:65]
            nc.vector.tensor_copy(ones_ap.rearrange("p k h x -> p k (h x)"),
                                  onesF[:])

            aexp_pool = tc.alloc_tile_pool(name="aexp_pool", bufs=3)
            # =============== phase A: U, rope tables, projections ===========
            with (
                tc.tile_pool(name="phA", bufs=1) as pa,
                tc.tile_pool(name="psA", bufs=1, space="PSUM") as psa,
            ):
                # --- one-hot U[axis*32+p, s] = (pos_axis[s] == p) ---
                # single cast-DMA: axis-major, each axis row replicated 32x
                posb = pa.tile([96, S], F32, name="posb")
                pos_rep = bass.AP(
                    tensor=pos3[:, :].tensor, offset=0,
                    ap=[[S, 3], [0, 32], [1, S]],
                )
                nc.gpsimd.dma_start(posb[:], pos_rep)
                nc.vector.tensor_scalar(
                    out=U[:, 0:S], in0=posb[:], scalar1=iot[:], scalar2=None,
                    op0=mybir.AluOpType.is_equal,
                )

                # --- cos/sin fields [128, S] via one-hot matmul ---
                rtab = pa.tile([96, 256], F32R, name="rtab")
                nc.sync.dma_start(rtab[:], ropetab[:])
                cosf = pa.tile([128, S], F32R, name="cosf")
                sinf = pa.tile([128, S], F32R, name="sinf")
                for ci, (q0, qw) in (() if "csaug" in ko
                                     else enumerate(CHUNKS)):
                    qe = qw + (qw % 2)
                    pcs = psa.tile([128, 512], F32, tag="mm", bufs=2,
                                   name=f"pcs{ci}")
                    nc.tensor.matmul(pcs[:, :qe], rtab[:, 0:128],
                                     U[:, q0 : q0 + qe], start=True, stop=True)
                    nc.scalar.copy(cosf[:, q0 : q0 + qw], pcs[:, :qw])
                    psn = psa.tile([128, 512], F32, tag="mm", bufs=2,
                                   name=f"psn{ci}")
                    nc.tensor.matmul(psn[:, :qe], rtab[:, 128:256],
                                     U[:, q0 : q0 + qe], start=True, stop=True)
                    nc.scalar.copy(sinf[:, q0 : q0 + qw], psn[:, :qw])

                # --- bias augmentation rows for q~ ---
                mg = pa.tile([96, HPC, 96], F32R, name="mg")
                nc.sync.dma_start(mg[:], maug[:])
                for h in (() if "csaug" in ko else range(HPC)):
                    for ci, (q0, qw) in enumerate(CHUNKS):
                        qe = qw + (qw % 2)
                        pag = psa.tile([128, 512], F32, tag="mm", bufs=2,
                                       name=f"pag{h}_{ci}")
                        nc.tensor.matmul(pag[0:96, :qe], mg[:, h, :],
                                         U[:, q0 : q0 + qe], start=True,
                                         stop=True)
                        nc.scalar.copy(qaug[:, h, q0 : q0 + qw],
                                       pag[0:96, :qw])

                # --- projections (per q-chunk), rope + v-transpose fused ---
                wp = pa.tile([128, 8, 384], F32R, name="wp")
                nc.scalar.dma_start(wp[:], wproj[:])
                with tc.tile_pool(name="pchunk", bufs=1) as pchk:
                    for ci, (q0, qw) in enumerate(CHUNKS):
                        qe = qw + (qw % 2)
                        xt = pchk.tile([128, 8, 512], F32R, tag="xt", bufs=2,
                                       name=f"xt{ci}")
                        for kt in range(8):
                            eng = nc.sync if kt % 2 == 0 else nc.scalar
                            eng.dma_start(
                                xt[:, kt, 0:qw],
                                xTd[128 * kt : 128 * kt + 128, q0 : q0 + qw])
                        if qe != qw:
                            nc.vector.tensor_copy(
                                xt[:, :, qw:qe].rearrange("p k x -> p (k x)"),
                                zcolF[:, 0:8])
                        outs = []
                        for mb, mbn in enumerate(["q2c", "k2c", "vTc"]):
                            ps = psa.tile([128, 512], F32, tag=f"pj{mb}",
                                          bufs=1, name=f"pj{mb}_{ci}")
                            for kt in range(8):
                                nc.tensor.matmul(
                                    ps[:, :qe],
                                    wp[:, kt, 128 * mb : 128 * mb + 128],
                                    xt[:, kt, 0:qe],
                                    start=(kt == 0), stop=(kt == 7),
                                )
                            dst = pchk.tile([128, 512], F32R, tag=mbn,
                                            bufs=2, name=f"{mbn}_{ci}")
                            if mb == 0:
                                nc.scalar.mul(dst[:, :qw], ps[:, :qw], SCALE)
                            else:
                                nc.vector.tensor_copy(dst[:, :qw], ps[:, :qw])
                            outs.append(dst)
                        q2c, k2c, vTc = outs
                        # rotate-half: rh[a] = -src[b], rh[b] = +src[a]
                        qrh2c = pchk.tile([128, 512], F32R, tag="qrh2c",
                                          bufs=2, name=f"qrh2c_{ci}")
                        krh2c = pchk.tile([128, 512], F32R, tag="krh2c",
                                          bufs=2, name=f"krh2c_{ci}")
                        for src, rh in ((q2c, qrh2c), (k2c, krh2c)):
                            for h in range(HPC):
                                a, b = 64 * h, 64 * h + 32
                                nc.gpsimd.tensor_scalar_mul(
                                    rh[a : a + 32, :qw],
                                    src[b : b + 32, :qw], -1.0)
                                nc.gpsimd.tensor_copy(
                                    rh[b : b + 32, :qw],
                                    src[a : a + 32, :qw])
                        # rope for this chunk
                        if "rope" in ko:
                            continue
                        tq = pchk.tile([128, 512], F32R, tag="tq", bufs=2,
                                       name=f"tq_{ci}")
                        nc.vector.tensor_mul(tq[:, :qw], qrh2c[:, :qw],
                                             sinf[:, q0 : q0 + qw])
                        nc.vector.tensor_mul(qr[:, q0 : q0 + qw], q2c[:, :qw],
                                             cosf[:, q0 : q0 + qw])
                        nc.vector.tensor_add(qr[:, q0 : q0 + qw],
                                             qr[:, q0 : q0 + qw], tq[:, :qw])
                        tk = pchk.tile([128, 512], F32R, tag="tk", bufs=2,
                                       name=f"tk_{ci}")
                        nc.gpsimd.tensor_mul(tk[:, :qw], krh2c[:, :qw],
                                             sinf[:, q0 : q0 + qw])
                        nc.gpsimd.tensor_mul(kr[:, q0 : q0 + qw], k2c[:, :qw],
                                             cosf[:, q0 : q0 + qw])
                        nc.gpsimd.tensor_add(kr[:, q0 : q0 + qw],
                                             kr[:, q0 : q0 + qw], tk[:, :qw])
                        # v transpose for k-tiles inside this chunk
                        for kt, (k0, kw) in enumerate(KTILES):
                            if not (q0 <= k0 < q0 + qw):
                                continue
                            off = k0 - q0
                            pvt = psa.tile([128, 512], F32R, tag="vt", bufs=1,
                                           name=f"pvt{kt}")
                            nc.tensor.transpose(pvt[0:kw, 0:128],
                                                vTc[:, off : off + kw],
                                                ident[:])
                            nc.vector.tensor_copy(
                                v_nat[0:kw, kt, :].rearrange(
                                    "p (h x) -> p h x", h=2)[:, :, 0:64],
                                pvt[0:kw, 0:128].rearrange(
                                    "p (h d) -> p h d", h=2),
                            )

            if debug:
                nc.sync.dma_start(dbg["d_U"][:], U[:, 0:S])
                nc.sync.dma_start(dbg["d_cosf"][:], cosf[:])
                nc.sync.dma_start(dbg["d_sinf"][:], sinf[:])
                nc.sync.dma_start(dbg["d_qr"][:], qr[:, 0:S])
                nc.sync.dma_start(dbg["d_kr"][:], kr[:, 0:S])
                nc.sync.dma_start(
                    dbg["d_qaug"][:].rearrange("p (h x) -> p h x", h=HPC),
                    qaug[:, :, 0:S])
                nc.sync.dma_start(
                    dbg["d_vnat"][:].rearrange("p (k x) -> p k x", k=NKT),
                    v_nat[:])

            if stop_after == "A":
                nc.sync.dma_start(outT[:], qr[:, 0:S].bitcast(F32))
            if stop_after is None or stop_after >= "B":
                # ======== phase B+C: attention, halved AG, out projection ==
                HOFF = [0, 1024]
                HWID = [1024, S - 1024]  # 1024 + 839
                HCHUNKS = [[0, 1], [2, 3]]
                agin_d = [dp.tile([128, HWID[i]], F32R, name=f"agin{i}")
                          for i in range(2)]
                agout_d = [dp.tile([NC * 128, HWID[i]], F32R,
                                   addr_space="Shared", name=f"agout{i}")
                           for i in range(2)]
                with (
                    tc.tile_pool(name="phB", bufs=1) as pb,
                    tc.tile_pool(name="psB", bufs=1, space="PSUM") as psb,
                ):
                    ap = aexp_pool
                    wo = pb.tile([128, 8, 128], F32R, name="wo")
                    nc.sync.dma_start(wo[:], wout[:])
                    attn_all = pb.tile([128, 8, SP], F32R, name="attn_all")
                    nc.vector.tensor_copy(
                        attn_all[:, :, S:SP].rearrange("p k x -> p (k x)"),
                        zcolF[:, 0:8])
                    outTs = pb.tile([128, S], F32, name="outTs")

                    for half in range(2):
                        for ci in HCHUNKS[half]:
                            q0, qw = CHUNKS[ci]
                            qe = qw + (qw % 2)
                            qend = q0 + qe
                            kts = [kt for kt, (k0, kw) in enumerate(KTILES)
                                   if k0 <= q0 + qw - 1]
                            for h in range(HPC):
                                po = psb.tile([65, 512], F32, tag="pv",
                                              bufs=2, name=f"po{h}_{ci}")
                                for kt in kts:
                                    k0, kw = KTILES[kt]
                                    # diagonal trim: skip cols left of k0
                                    qlo = max(q0, min(k0, qend - 256))
                                    N = qend - qlo
                                    pss = psb.tile(
                                        [128, 512], F32, tag="sc", bufs=4,
                                        name=f"ps{h}_{ci}_{kt}")
                                    nc.tensor.matmul(
                                        pss[0:kw, :N],
                                        kr[64 * h : 64 * h + 64,
                                           k0 : k0 + kw],
                                        qr[64 * h : 64 * h + 64, qlo:qend],
                                        start=True, stop=False,
                                    )
                                    nc.tensor.matmul(
                                        pss[0:kw, :N],
                                        U[:, k0 : k0 + kw],
                                        qaug[:, h, qlo:qend],
                                        start=False, stop=True,
                                    )
                                    aexp = ap.tile([128, 512], F32R,
                                                   tag="ax",
                                                   name=f"ax{h}_{ci}_{kt}")
                                    nc.scalar.activation(
                                        aexp[0:kw, :N], pss[0:kw, :N],
                                        mybir.ActivationFunctionType.Exp,
                                    )
                                    if k0 + kw - 1 > qlo:
                                        W = min(q0 + qw, k0 + kw - 1) - qlo
                                        nc.gpsimd.affine_select(
                                            out=aexp[0:kw, 0:W],
                                            in_=aexp[0:kw, 0:W],
                                            compare_op=mybir.AluOpType.is_ge,
                                            fill=0.0, base=qlo - k0,
                                            channel_multiplier=-1,
                                            pattern=[[1, W]],
                                        )
                                    nc.tensor.matmul(
                                        po[:, qlo - q0 : qe],
                                        v_nat[0:kw, kt,
                                              65 * h : 65 * h + 65],
                                        aexp[0:kw, :N],
                                        start=(kt == kts[0]),
                                        stop=(kt == kts[-1]),
                                    )
                                nc.vector.tensor_copy(
                                    attnh[h][:, q0 : q0 + qw], po[:, :qw])

                        # normalize this half and start its all-gather
                        c0, cw = HOFF[half], HWID[half]
                        for h in range(HPC):
                            nc.vector.reciprocal(
                                zrecs[h][:, c0 : c0 + cw],
                                attnh[h][64:65, c0 : c0 + cw])
                            nc.gpsimd.partition_broadcast(
                                zbs[h][:, c0 : c0 + cw],
                                zrecs[h][:, c0 : c0 + cw])
                            nc.vector.tensor_mul(
                                agin_s[64 * h : 64 * h + 64, c0 : c0 + cw],
                                attnh[h][0:64, c0 : c0 + cw],
                                zbs[h][:, c0 : c0 + cw])
                        nc.sync.dma_start(agin_d[half][:],
                                          agin_s[:, c0 : c0 + cw])
                        if no_cc:
                            rep = bass.AP(
                                tensor=agin_d[half].tensor,
                                offset=agin_d[half].offset,
                                ap=[[0, 8], [cw, 128], [1, cw]],
                            )
                            nc.sync.dma_start(agout_d[half][:], rep)
                        else:
                            nc.gpsimd.collective_compute(
                                "AllGather",
                                mybir.AluOpType.bypass,
                                ins=[agin_d[half].opt()],
                                outs=[agout_d[half].opt()],
                                replica_groups=[list(range(NC))],
                            )
                        # load gathered half and run its out-projection
                        for g, eng in ((0, nc.sync), (1, nc.scalar)):
                            ag_rep = bass.AP(
                                tensor=agout_d[half].tensor,
                                offset=agout_d[half].offset + g * 4 * 128 * cw,
                                ap=[[cw, 128], [128 * cw, 4], [1, cw]],
                            )
                            eng.dma_start(
                                attn_all[:, 4 * g : 4 * g + 4, c0 : c0 + cw],
                                ag_rep)
                        for ci in HCHUNKS[half]:
                            q0, qw = CHUNKS[ci]
                            qe = qw + (qw % 2)
                            pso = psb.tile([128, 512], F32, tag="op",
                                           bufs=2, name=f"pso{ci}")
                            for kt in range(8):
                                nc.tensor.matmul(
                                    pso[:, :qe], wo[:, kt, :],
                                    attn_all[:, kt, q0 : q0 + qe],
                                    start=(kt == 0), stop=(kt == 7),
                                )
                            nc.vector.tensor_copy(outTs[:, q0 : q0 + qw],
                                                  pso[:, :qw])
                            nc.sync.dma_start(outT[:, q0 : q0 + qw],
                                              outTs[:, q0 : q0 + qw])

                    if debug:
                        nc.sync.dma_start(dbg["d_a0"][:], attnh[0][:])
                        nc.sync.dma_start(dbg["d_a1"][:], attnh[1][:])
                        nc.sync.dma_start(dbg["d_agin"][:], agin_s[:].bitcast(F32))
                        nc.sync.dma_start(
                            dbg["d_call"][:].rearrange("p (k x) -> p k x",
                                                       k=8),
                            attn_all[:, :, 0:S])
            if stop_after == "B":
                nc.sync.dma_start(outT[:], agin_s[:].bitcast(F32))
            aexp_pool.release()
    nc.compile()
    return nc


def _host_prep(x, Wqkv, Wout, bias_x, bias_y, bias_z, pos_xyz):
    """Shard/relayout inputs for the 8 cores. Pure relayout + constant tables."""
    xT = np.ascontiguousarray(x[0].T).astype(np.float32)  # [1024, S]
    pos3 = np.ascontiguousarray(pos_xyz[0].T).astype(np.int32)  # [3, S]

    # rope tables: lhsT [96, 128]; rows u=(axis,p), cols r (t = r % 32)
    invf = np.concatenate([
        BASE ** (-np.arange(0, 2 * PX, 2) / (2 * PX)),
        BASE ** (-np.arange(0, 2 * PY, 2) / (2 * PY)),
        BASE ** (-np.arange(0, 2 * PZ, 2) / (2 * PZ)),
    ]).astype(np.float64)  # [32]
    axis_of_t = np.array([0] * PX + [1] * PY + [2] * PZ)
    ropetab = np.zeros((96, 256), dtype=np.float32)
    p = np.arange(32)
    for r in range(128):
        t = r % 32
        ax = axis_of_t[t]
        p_eff = np.minimum(p, 7) if ax == 2 else p
        ang = p_eff * invf[t]
        ropetab[32 * ax : 32 * ax + 32, r] = np.cos(ang)
        ropetab[32 * ax : 32 * ax + 32, 128 + r] = np.sin(ang)

    iota96 = (np.arange(96) % 32).astype(np.float32).reshape(96, 1)

    # bias M matrices: M_ax[p_q, p_k] = bias_ax[clip(p_q - p_k, -c, c) + c, h]
    pq, pk = np.meshgrid(np.arange(32), np.arange(32), indexing="ij")
    dxy = np.clip(pq - pk, -MAX_DXY, MAX_DXY) + MAX_DXY
    dz = np.clip(pq - pk, -MAX_DZ, MAX_DZ) + MAX_DZ

    in_maps = []
    for c in range(NC):
        wblocks = []
        for blk in range(2):  # q-side then k-side
            wq = []
            for h in (HPC * c, HPC * c + 1):
                W = Wqkv[blk * D + HD * h : blk * D + HD * h + HD, :]  # [64,1024]
                wq.append(W[PERM])
            wblocks.append(np.concatenate(wq, axis=0))   # [128, 1024]
        wv = np.concatenate(
            [Wqkv[2 * D + HD * h : 2 * D + HD * h + HD, :]
             for h in (HPC * c, HPC * c + 1)], axis=0)
        wblocks.append(wv)
        # [384, 1024] -> lhsT layout [128(p within ktile), 8(ktile), 384(m)]
        wcat = np.concatenate(wblocks, axis=0)  # [384, 1024]
        wproj = np.ascontiguousarray(
            wcat.T.reshape(8, 128, 384).transpose(1, 0, 2)).astype(np.float32)

        woutT = Wout[128 * c : 128 * c + 128, :].T  # [1024, 128]
        wout_l = np.ascontiguousarray(
            woutT.reshape(8, 128, 128).transpose(1, 0, 2)).astype(np.float32)

        mg = np.zeros((96, HPC, 96), dtype=np.float32)
        for i, h in enumerate((HPC * c, HPC * c + 1)):
            mg[0:32, i, 0:32] = bias_x[dxy, h]
            mg[32:64, i, 32:64] = bias_y[dxy, h]
            mg[64:96, i, 64:96] = bias_z[dz, h]

        in_maps.append({
            "xT": xT, "wproj": wproj, "wout": wout_l, "pos3": pos3,
            "ropetab": ropetab, "maug": mg, "iota96": iota96,
        })
    return in_maps


_NC_CACHE = {}


def kernel(x, Wqkv, Wout, bias_x, bias_y, bias_z, pos_xyz, _bench=[None]):
    x = np.asarray(x, dtype=np.float32)
    Wqkv = np.asarray(Wqkv, dtype=np.float32)
    Wout = np.asarray(Wout, dtype=np.float32)
    bias_x = np.asarray(bias_x, dtype=np.float32)
    bias_y = np.asarray(bias_y, dtype=np.float32)
    bias_z = np.asarray(bias_z, dtype=np.float32)
    pos_xyz = np.asarray(pos_xyz, dtype=np.int32)

    in_maps = _host_prep(x, Wqkv, Wout, bias_x, bias_y, bias_z, pos_xyz)
    if "nc" not in _NC_CACHE:
        _NC_CACHE["nc"] = _build_nc()
    nc = _NC_CACHE["nc"]
    res = run_bass_kernel_spmd(nc, in_maps, core_ids=list(range(NC)))
    _bench[0] = res
    outT = np.concatenate([res.results[c]["outT"] for c in range(NC)], axis=0)
    return np.ascontiguousarray(outT.T).reshape(B, S, D).astype(np.float32)


# revision 41
# speedup vs baseline: 1.0856x; 1.0856x over previous
"""MultiHeadSelfAttention with 3D RoPE + 3D relative position bias on 8 trn2 cores.

Sharding: tensor-parallel over heads (2 heads per core). Each core:
  - projects q/k/v for its 2 heads (plus rotate-half projections for RoPE),
  - computes scores^T = k~.q~ with the 3D relative bias folded into the
    matmul contraction via one-hot position encodings,
  - causal-masked unnormalized-exp softmax (scores bounded, no max needed),
  - PV matmul with an appended ones-column producing softmax denominators,
  - AllGather of per-head attention outputs, then a row-slice of the final
    output projection.
Host does only sharding/relayout of inputs and the final gather/transpose.
"""
import numpy as np

import concourse.bass as bass
import concourse.bacc as bacc
import concourse.tile as tile
import concourse.mybir as mybir
from concourse.bass_utils import run_bass_kernel_spmd
from concourse.masks import make_identity

B, S, D, H, HD = 1, 1863, 1024, 16, 64
NC = 8
HPC = H // NC  # heads per core = 2
BASE = 10000.0
MAX_DXY, MAX_DZ = 30, 8
PX = PY = 10
PZ = 12
SCALE = HD ** -0.5

F32 = mybir.dt.float32
F32R = mybir.dt.float32r
I32 = mybir.dt.int32

# q-chunks (PSUM bank = 512 fp32) and k-tiles (128 partitions)
CHUNKS = [(c * 512, min(512, S - c * 512)) for c in range((S + 511) // 512)]
KTILES = [(kt * 128, min(128, S - kt * 128)) for kt in range((S + 127) // 128)]
NKT = len(KTILES)  # 15
SP = S + 1  # f32r matmuls need even moving-dim N; pad rhs tensors to 1864
PERM = np.concatenate([np.arange(0, HD, 2), np.arange(1, HD, 2)])  # even, odd


def _build_nc(debug=False, stop_after=None, no_cc=False, ko=()):
    nc = bacc.Bacc("TRN2")

    xTd = nc.dram_tensor("xT", [D, S], F32R, kind="ExternalInput")
    wproj = nc.dram_tensor("wproj", [128, 8, 384], F32R, kind="ExternalInput")
    wout = nc.dram_tensor("wout", [128, 8, 128], F32R, kind="ExternalInput")
    pos3 = nc.dram_tensor("pos3", [3, S], I32, kind="ExternalInput")
    ropetab = nc.dram_tensor("ropetab", [96, 256], F32R, kind="ExternalInput")
    maug = nc.dram_tensor("maug", [96, HPC, 96], F32R, kind="ExternalInput")
    iota96 = nc.dram_tensor("iota96", [96, 1], F32, kind="ExternalInput")
    outT = nc.dram_tensor("outT", [128, S], F32, kind="ExternalOutput")
    dbg = {}
    if debug:
        for nm, shp in [("d_U", [96, S]), ("d_cosf", [128, S]),
                        ("d_sinf", [128, S]), ("d_qr", [128, S]),
                        ("d_kr", [128, S]), ("d_qaug", [96, HPC * S]),
                        ("d_vnat", [128, NKT * 130]),
                        ("d_call", [128, 8 * S])]:
            dbg[nm] = nc.dram_tensor(nm, shp, F32R, kind="ExternalOutput")
        for nm, shp in [("d_a0", [65, S]), ("d_a1", [65, S]),
                        ("d_agin", [128, S])]:
            dbg[nm] = nc.dram_tensor(nm, shp, F32, kind="ExternalOutput")

    with tile.TileContext(nc) as tc:
        with (
            tc.tile_pool(name="persist", bufs=1) as pp,
            tc.tile_pool(name="dram", bufs=1, space="DRAM") as dp,
        ):
            # ---- persistent tiles (live across phases) ----
            U = pp.tile([96, SP], F32R, name="U")
            qr = pp.tile([128, SP], F32R, name="qr")
            kr = pp.tile([128, SP], F32R, name="kr")
            qaug = pp.tile([96, HPC, SP], F32R, name="qaug")
            v_nat = pp.tile([128, NKT, 130], F32R, name="v_nat")
            ident = pp.tile([128, 128], F32R, name="ident")
            iot = pp.tile([96, 1], F32, name="iot")
            agin_s = pp.tile([128, S], F32R, name="agin_s")

            nc.sync.dma_start(iot[:], iota96[:])
            zcolF = pp.tile([128, 8], F32, name="zcolF")
            nc.gpsimd.memset(zcolF[:], 0.0)
            nc.vector.tensor_copy(U[:, S:SP], zcolF[0:96, 0:1])
            nc.vector.tensor_copy(qr[:, S:SP], zcolF[:, 0:1])
            nc.vector.tensor_copy(kr[:, S:SP], zcolF[:, 0:1])
            nc.vector.tensor_copy(
                qaug[:, :, S:SP].rearrange("p h x -> p (h x)"),
                zcolF[0:96, 0:2])
            attnh = [pp.tile([65, S], F32, name=f"attnh{h}")
                     for h in range(HPC)]
            zrecs = [pp.tile([1, S], F32, name=f"zrec{h}")
                     for h in range(HPC)]
            zbs = [pp.tile([64, S], F32, name=f"zb{h}") for h in range(HPC)]
            identF = pp.tile([128, 128], F32, name="identF")
            make_identity(nc, identF[:])
            nc.vector.tensor_copy(ident[:], identF[:])
            # ones columns of v_nat (cols 64 and 129 of each k-tile block)
            onesF = pp.tile([128, NKT, 2], F32, name="onesF")
            nc.gpsimd.memset(onesF[:], 1.0)
            ones_ap = v_nat[:].rearrange("p k (h x) -> p k h x", h=2)[:, :, :, 64:65]
            nc.vector.tensor_copy(ones_ap.rearrange("p k h x -> p k (h x)"),
                                  onesF[:])

            aexp_pool = tc.alloc_tile_pool(name="aexp_pool", bufs=3)
            # =============== phase A: U, rope tables, projections ===========
            with (
                tc.tile_pool(name="phA", bufs=1) as pa,
                tc.tile_pool(name="psA", bufs=1, space="PSUM") as psa,
            ):
                # --- one-hot U[axis*32+p, s] = (pos_axis[s] == p) ---
                # single cast-DMA: axis-major, each axis row replicated 32x
                posb = pa.tile([96, S], F32, name="posb")
                pos_rep = bass.AP(
                    tensor=pos3[:, :].tensor, offset=0,
                    ap=[[S, 3], [0, 32], [1, S]],
                )
                nc.gpsimd.dma_start(posb[:], pos_rep)
                nc.vector.tensor_scalar(
                    out=U[:, 0:S], in0=posb[:], scalar1=iot[:], scalar2=None,
                    op0=mybir.AluOpType.is_equal,
                )

                # --- cos/sin fields [128, S] via one-hot matmul ---
                rtab = pa.tile([96, 256], F32R, name="rtab")
                nc.sync.dma_start(rtab[:], ropetab[:])
                cosf = pa.tile([128, S], F32R, name="cosf")
                sinf = pa.tile([128, S], F32R, name="sinf")
                for ci, (q0, qw) in (() if "csaug" in ko
                                     else enumerate(CHUNKS)):
                    qe = qw + (qw % 2)
                    pcs = psa.tile([128, 512], F32, tag="mm", bufs=2,
                                   name=f"pcs{ci}")
                    nc.tensor.matmul(pcs[:, :qe], rtab[:, 0:128],
                                     U[:, q0 : q0 + qe], start=True, stop=True)
                    nc.scalar.copy(cosf[:, q0 : q0 + qw], pcs[:, :qw])
                    psn = psa.tile([128, 512], F32, tag="mm", bufs=2,
                                   name=f"psn{ci}")
                    nc.tensor.matmul(psn[:, :qe], rtab[:, 128:256],
                                     U[:, q0 : q0 + qe], start=True, stop=True)
                    nc.scalar.copy(sinf[:, q0 : q0 + qw], psn[:, :qw])

                # --- bias augmentation rows for q~ ---
                mg = pa.tile([96, HPC, 96], F32R, name="mg")
                nc.sync.dma_start(mg[:], maug[:])
                for h in (() if "csaug" in ko else range(HPC)):
                    for ci, (q0, qw) in enumerate(CHUNKS):
                        qe = qw + (qw % 2)
                        pag = psa.tile([128, 512], F32, tag="mm", bufs=2,
                                       name=f"pag{h}_{ci}")
                        nc.tensor.matmul(pag[0:96, :qe], mg[:, h, :],
                                         U[:, q0 : q0 + qe], start=True,
                                         stop=True)
                        nc.scalar.copy(qaug[:, h, q0 : q0 + qw],
                                       pag[0:96, :qw])

                # --- projections (per q-chunk), rope + v-transpose fused ---
                wp = pa.tile([128, 8, 384], F32R, name="wp")
                nc.scalar.dma_start(wp[:], wproj[:])
                with tc.tile_pool(name="pchunk", bufs=1) as pchk:
                    for ci, (q0, qw) in enumerate(CHUNKS):
                        qe = qw + (qw % 2)
                        xt = pchk.tile([128, 8, 512], F32R, tag="xt", bufs=2,
                                       name=f"xt{ci}")
                        for kt in range(8):
                            eng = nc.sync if kt % 2 == 0 else nc.scalar
                            eng.dma_start(
                                xt[:, kt, 0:qw],
                                xTd[128 * kt : 128 * kt + 128, q0 : q0 + qw])
                        if qe != qw:
                            nc.vector.tensor_copy(
                                xt[:, :, qw:qe].rearrange("p k x -> p (k x)"),
                                zcolF[:, 0:8])
                        outs = []
                        for mb, mbn in enumerate(["q2c", "k2c", "vTc"]):
                            ps = psa.tile([128, 512], F32, tag=f"pj{mb}",
                                          bufs=1, name=f"pj{mb}_{ci}")
                            for kt in range(8):
                                nc.tensor.matmul(
                                    ps[:, :qe],
                                    wp[:, kt, 128 * mb : 128 * mb + 128],
                                    xt[:, kt, 0:qe],
                                    start=(kt == 0), stop=(kt == 7),
                                )
                            dst = pchk.tile([128, 512], F32R, tag=mbn,
                                            bufs=2, name=f"{mbn}_{ci}")
                            if mb == 0:
                                nc.scalar.mul(dst[:, :qw], ps[:, :qw], SCALE)
                            else:
                                nc.vector.tensor_copy(dst[:, :qw], ps[:, :qw])
                            outs.append(dst)
                        q2c, k2c, vTc = outs
                        # rotate-half: rh[a] = -src[b], rh[b] = +src[a]
                        qrh2c = pchk.tile([128, 512], F32R, tag="qrh2c",
                                          bufs=2, name=f"qrh2c_{ci}")
                        krh2c = pchk.tile([128, 512], F32R, tag="krh2c",
                                          bufs=2, name=f"krh2c_{ci}")
                        for src, rh in ((q2c, qrh2c), (k2c, krh2c)):
                            for h in range(HPC):
                                a, b = 64 * h, 64 * h + 32
                                nc.gpsimd.tensor_scalar_mul(
                                    rh[a : a + 32, :qw],
                                    src[b : b + 32, :qw], -1.0)
                                nc.gpsimd.tensor_copy(
                                    rh[b : b + 32, :qw],
                                    src[a : a + 32, :qw])
                        # rope for this chunk
                        if "rope" in ko:
                            continue
                        tq = pchk.tile([128, 512], F32R, tag="tq", bufs=2,
                                       name=f"tq_{ci}")
                        nc.vector.tensor_mul(tq[:, :qw], qrh2c[:, :qw],
                                             sinf[:, q0 : q0 + qw])
                        nc.vector.tensor_mul(qr[:, q0 : q0 + qw], q2c[:, :qw],
                                             cosf[:, q0 : q0 + qw])
                        nc.vector.tensor_add(qr[:, q0 : q0 + qw],
                                             qr[:, q0 : q0 + qw], tq[:, :qw])
                        tk = pchk.tile([128, 512], F32R, tag="tk", bufs=2,
                                       name=f"tk_{ci}")
                        nc.gpsimd.tensor_mul(tk[:, :qw], krh2c[:, :qw],
                                             sinf[:, q0 : q0 + qw])
                        nc.gpsimd.tensor_mul(kr[:, q0 : q0 + qw], k2c[:, :qw],
                                             cosf[:, q0 : q0 + qw])
                        nc.gpsimd.tensor_add(kr[:, q0 : q0 + qw],
                                             kr[:, q0 : q0 + qw], tk[:, :qw])
                        # v transpose for k-tiles inside this chunk
                        for kt, (k0, kw) in enumerate(KTILES):
                            if not (q0 <= k0 < q0 + qw):
                                continue
                            off = k0 - q0
                            pvt = psa.tile([128, 512], F32R, tag="vt", bufs=1,
                                           name=f"pvt{kt}")
                            nc.tensor.transpose(pvt[0:kw, 0:128],
                                                vTc[:, off : off + kw],
                                                ident[:])
                            nc.vector.tensor_copy(
                                v_nat[0:kw, kt, :].rearrange(
                                    "p (h x) -> p h x", h=2)[:, :, 0:64],
                                pvt[0:kw, 0:128].rearrange(
                                    "p (h d) -> p h d", h=2),
                            )

            if debug:
                nc.sync.dma_start(dbg["d_U"][:], U[:, 0:S])
                nc.sync.dma_start(dbg["d_cosf"][:], cosf[:])
                nc.sync.dma_start(dbg["d_sinf"][:], sinf[:])
                nc.sync.dma_start(dbg["d_qr"][:], qr[:, 0:S])
                nc.sync.dma_start(dbg["d_kr"][:], kr[:, 0:S])
                nc.sync.dma_start(
                    dbg["d_qaug"][:].rearrange("p (h x) -> p h x", h=HPC),
                    qaug[:, :, 0:S])
                nc.sync.dma_start(
                    dbg["d_vnat"][:].rearrange("p (k x) -> p k x", k=NKT),
                    v_nat[:])

            if stop_after == "A":
                nc.sync.dma_start(outT[:], qr[:, 0:S].bitcast(F32))
            if stop_after is None or stop_after >= "B":
                # ======== phase B+C: attention, halved AG, out projection ==
                # asymmetric split: AG0 (chunks 0-2) overlaps chunk 3's
                # attention (the longest); AG1 is only 327 cols of tail
                HOFF = [0, 1536]
                HWID = [1536, S - 1536]  # 1536 + 327
                HCHUNKS = [[0, 1, 2], [3]]
                agin_d = [dp.tile([128, HWID[i]], F32R, name=f"agin{i}")
                          for i in range(2)]
                agout_d = [dp.tile([NC * 128, HWID[i]], F32R,
                                   addr_space="Shared", name=f"agout{i}")
                           for i in range(2)]
                with (
                    tc.tile_pool(name="phB", bufs=1) as pb,
                    tc.tile_pool(name="psB", bufs=1, space="PSUM") as psb,
                ):
                    ap = aexp_pool
                    wo = pb.tile([128, 8, 128], F32R, name="wo")
                    nc.sync.dma_start(wo[:], wout[:])
                    attn_all = pb.tile([128, 8, SP], F32R, name="attn_all")
                    nc.vector.tensor_copy(
                        attn_all[:, :, S:SP].rearrange("p k x -> p (k x)"),
                        zcolF[:, 0:8])
                    outTs = pb.tile([128, S], F32, name="outTs")

                    for half in range(2):
                        for ci in HCHUNKS[half]:
                            q0, qw = CHUNKS[ci]
                            qe = qw + (qw % 2)
                            qend = q0 + qe
                            kts = [kt for kt, (k0, kw) in enumerate(KTILES)
                                   if k0 <= q0 + qw - 1]
                            for h in range(HPC):
                                po = psb.tile([65, 512], F32, tag="pv",
                                              bufs=2, name=f"po{h}_{ci}")
                                for kt in kts:
                                    k0, kw = KTILES[kt]
                                    # diagonal trim: skip cols left of k0
                                    qlo = max(q0, min(k0, qend - 256))
                                    N = qend - qlo
                                    pss = psb.tile(
                                        [128, 512], F32, tag="sc", bufs=4,
                                        name=f"ps{h}_{ci}_{kt}")
                                    nc.tensor.matmul(
                                        pss[0:kw, :N],
                                        kr[64 * h : 64 * h + 64,
                                           k0 : k0 + kw],
                                        qr[64 * h : 64 * h + 64, qlo:qend],
                                        start=True, stop=False,
                                    )
                                    nc.tensor.matmul(
                                        pss[0:kw, :N],
                                        U[:, k0 : k0 + kw],
                                        qaug[:, h, qlo:qend],
                                        start=False, stop=True,
                                    )
                                    aexp = ap.tile([128, 512], F32R,
                                                   tag="ax",
                                                   name=f"ax{h}_{ci}_{kt}")
                                    nc.scalar.activation(
                                        aexp[0:kw, :N], pss[0:kw, :N],
                                        mybir.ActivationFunctionType.Exp,
                                    )
                                    if k0 + kw - 1 > qlo:
                                        W = min(q0 + qw, k0 + kw - 1) - qlo
                                        nc.gpsimd.affine_select(
                                            out=aexp[0:kw, 0:W],
                                            in_=aexp[0:kw, 0:W],
                                            compare_op=mybir.AluOpType.is_ge,
                                            fill=0.0, base=qlo - k0,
                                            channel_multiplier=-1,
                                            pattern=[[1, W]],
                                        )
                                    nc.tensor.matmul(
                                        po[:, qlo - q0 : qe],
                                        v_nat[0:kw, kt,
                                              65 * h : 65 * h + 65],
                                        aexp[0:kw, :N],
                                        start=(kt == kts[0]),
                                        stop=(kt == kts[-1]),
                                    )
                                nc.vector.tensor_copy(
                                    attnh[h][:, q0 : q0 + qw], po[:, :qw])

                        # normalize this half and start its all-gather
                        c0, cw = HOFF[half], HWID[half]
                        for h in range(HPC):
                            nc.vector.reciprocal(
                                zrecs[h][:, c0 : c0 + cw],
                                attnh[h][64:65, c0 : c0 + cw])
                            nc.gpsimd.partition_broadcast(
                                zbs[h][:, c0 : c0 + cw],
                                zrecs[h][:, c0 : c0 + cw])
                            nc.vector.tensor_mul(
                                agin_s[64 * h : 64 * h + 64, c0 : c0 + cw],
                                attnh[h][0:64, c0 : c0 + cw],
                                zbs[h][:, c0 : c0 + cw])
                        nc.sync.dma_start(agin_d[half][:],
                                          agin_s[:, c0 : c0 + cw])
                        if no_cc:
                            rep = bass.AP(
                                tensor=agin_d[half].tensor,
                                offset=agin_d[half].offset,
                                ap=[[0, 8], [cw, 128], [1, cw]],
                            )
                            nc.sync.dma_start(agout_d[half][:], rep)
                        else:
                            nc.gpsimd.collective_compute(
                                "AllGather",
                                mybir.AluOpType.bypass,
                                ins=[agin_d[half].opt()],
                                outs=[agout_d[half].opt()],
                                replica_groups=[list(range(NC))],
                            )
                        # load gathered half and run its out-projection
                        for g, eng in ((0, nc.sync), (1, nc.scalar)):
                            ag_rep = bass.AP(
                                tensor=agout_d[half].tensor,
                                offset=agout_d[half].offset + g * 4 * 128 * cw,
                                ap=[[cw, 128], [128 * cw, 4], [1, cw]],
                            )
                            eng.dma_start(
                                attn_all[:, 4 * g : 4 * g + 4, c0 : c0 + cw],
                                ag_rep)
                        for ci in HCHUNKS[half]:
                            q0, qw = CHUNKS[ci]
                            qe = qw + (qw % 2)
                            pso = psb.tile([128, 512], F32, tag="op",
                                           bufs=2, name=f"pso{ci}")
                            for kt in range(8):
                                nc.tensor.matmul(
                                    pso[:, :qe], wo[:, kt, :],
                                    attn_all[:, kt, q0 : q0 + qe],
                                    start=(kt == 0), stop=(kt == 7),
                                )
                            nc.vector.tensor_copy(outTs[:, q0 : q0 + qw],
                                                  pso[:, :qw])
                            nc.sync.dma_start(outT[:, q0 : q0 + qw],
                                              outTs[:, q0 : q0 + qw])

                    if debug:
                        nc.sync.dma_start(dbg["d_a0"][:], attnh[0][:])
                        nc.sync.dma_start(dbg["d_a1"][:], attnh[1][:])
                        nc.sync.dma_start(dbg["d_agin"][:], agin_s[:].bitcast(F32))
                        nc.sync.dma_start(
                            dbg["d_call"][:].rearrange("p (k x) -> p k x",
                                                       k=8),
                            attn_all[:, :, 0:S])
            if stop_after == "B":
                nc.sync.dma_start(outT[:], agin_s[:].bitcast(F32))
            aexp_pool.release()
    nc.compile()
    return nc


def _host_prep(x, Wqkv, Wout, bias_x, bias_y, bias_z, pos_xyz):
    """Shard/relayout inputs for the 8 cores. Pure relayout + constant tables."""
    xT = np.ascontiguousarray(x[0].T).astype(np.float32)  # [1024, S]
    pos3 = np.ascontiguousarray(pos_xyz[0].T).astype(np.int32)  # [3, S]

    # rope tables: lhsT [96, 128]; rows u=(axis,p), cols r (t = r % 32)
    invf = np.concatenate([
        BASE ** (-np.arange(0, 2 * PX, 2) / (2 * PX)),
        BASE ** (-np.arange(0, 2 * PY, 2) / (2 * PY)),
        BASE ** (-np.arange(0, 2 * PZ, 2) / (2 * PZ)),
    ]).astype(np.float64)  # [32]
    axis_of_t = np.array([0] * PX + [1] * PY + [2] * PZ)
    ropetab = np.zeros((96, 256), dtype=np.float32)
    p = np.arange(32)
    for r in range(128):
        t = r % 32
        ax = axis_of_t[t]
        p_eff = np.minimum(p, 7) if ax == 2 else p
        ang = p_eff * invf[t]
        ropetab[32 * ax : 32 * ax + 32, r] = np.cos(ang)
        ropetab[32 * ax : 32 * ax + 32, 128 + r] = np.sin(ang)

    iota96 = (np.arange(96) % 32).astype(np.float32).reshape(96, 1)

    # bias M matrices: M_ax[p_q, p_k] = bias_ax[clip(p_q - p_k, -c, c) + c, h]
    pq, pk = np.meshgrid(np.arange(32), np.arange(32), indexing="ij")
    dxy = np.clip(pq - pk, -MAX_DXY, MAX_DXY) + MAX_DXY
    dz = np.clip(pq - pk, -MAX_DZ, MAX_DZ) + MAX_DZ

    in_maps = []
    for c in range(NC):
        wblocks = []
        for blk in range(2):  # q-side then k-side
            wq = []
            for h in (HPC * c, HPC * c + 1):
                W = Wqkv[blk * D + HD * h : blk * D + HD * h + HD, :]  # [64,1024]
                wq.append(W[PERM])
            wblocks.append(np.concatenate(wq, axis=0))   # [128, 1024]
        wv = np.concatenate(
            [Wqkv[2 * D + HD * h : 2 * D + HD * h + HD, :]
             for h in (HPC * c, HPC * c + 1)], axis=0)
        wblocks.append(wv)
        # [384, 1024] -> lhsT layout [128(p within ktile), 8(ktile), 384(m)]
        wcat = np.concatenate(wblocks, axis=0)  # [384, 1024]
        wproj = np.ascontiguousarray(
            wcat.T.reshape(8, 128, 384).transpose(1, 0, 2)).astype(np.float32)

        woutT = Wout[128 * c : 128 * c + 128, :].T  # [1024, 128]
        wout_l = np.ascontiguousarray(
            woutT.reshape(8, 128, 128).transpose(1, 0, 2)).astype(np.float32)

        mg = np.zeros((96, HPC, 96), dtype=np.float32)
        for i, h in enumerate((HPC * c, HPC * c + 1)):
            mg[0:32, i, 0:32] = bias_x[dxy, h]
            mg[32:64, i, 32:64] = bias_y[dxy, h]
            mg[64:96, i, 64:96] = bias_z[dz, h]

        in_maps.append({
            "xT": xT, "wproj": wproj, "wout": wout_l, "pos3": pos3,
            "ropetab": ropetab, "maug": mg, "iota96": iota96,
        })
    return in_maps


_NC_CACHE = {}


def kernel(x, Wqkv, Wout, bias_x, bias_y, bias_z, pos_xyz, _bench=[None]):
    x = np.asarray(x, dtype=np.float32)
    Wqkv = np.asarray(Wqkv, dtype=np.float32)
    Wout = np.asarray(Wout, dtype=np.float32)
    bias_x = np.asarray(bias_x, dtype=np.float32)
    bias_y = np.asarray(bias_y, dtype=np.float32)
    bias_z = np.asarray(bias_z, dtype=np.float32)
    pos_xyz = np.asarray(pos_xyz, dtype=np.int32)

    in_maps = _host_prep(x, Wqkv, Wout, bias_x, bias_y, bias_z, pos_xyz)
    if "nc" not in _NC_CACHE:
        _NC_CACHE["nc"] = _build_nc()
    nc = _NC_CACHE["nc"]
    res = run_bass_kernel_spmd(nc, in_maps, core_ids=list(range(NC)))
    _bench[0] = res
    outT = np.concatenate([res.results[c]["outT"] for c in range(NC)], axis=0)
    return np.ascontiguousarray(outT.T).reshape(B, S, D).astype(np.float32)


# revision 44
# speedup vs baseline: 1.0956x; 1.0092x over previous
"""MultiHeadSelfAttention with 3D RoPE + 3D relative position bias on 8 trn2 cores.

Sharding: tensor-parallel over heads (2 heads per core). Each core:
  - projects q/k/v for its 2 heads (plus rotate-half projections for RoPE),
  - computes scores^T = k~.q~ with the 3D relative bias folded into the
    matmul contraction via one-hot position encodings,
  - causal-masked unnormalized-exp softmax (scores bounded, no max needed),
  - PV matmul with an appended ones-column producing softmax denominators,
  - AllGather of per-head attention outputs, then a row-slice of the final
    output projection.
Host does only sharding/relayout of inputs and the final gather/transpose.
"""
import numpy as np

import concourse.bass as bass
import concourse.bacc as bacc
import concourse.tile as tile
import concourse.mybir as mybir
from concourse.bass_utils import run_bass_kernel_spmd
from concourse.masks import make_identity

B, S, D, H, HD = 1, 1863, 1024, 16, 64
NC = 8
HPC = H // NC  # heads per core = 2
BASE = 10000.0
MAX_DXY, MAX_DZ = 30, 8
PX = PY = 10
PZ = 12
SCALE = HD ** -0.5

F32 = mybir.dt.float32
F32R = mybir.dt.float32r
I32 = mybir.dt.int32

# q-chunks (PSUM bank = 512 fp32) and k-tiles (128 partitions)
CHUNKS = [(c * 512, min(512, S - c * 512)) for c in range((S + 511) // 512)]
KTILES = [(kt * 128, min(128, S - kt * 128)) for kt in range((S + 127) // 128)]
NKT = len(KTILES)  # 15
SP = S + 1  # f32r matmuls need even moving-dim N; pad rhs tensors to 1864
PERM = np.concatenate([np.arange(0, HD, 2), np.arange(1, HD, 2)])  # even, odd


def _build_nc(debug=False, stop_after=None, no_cc=False, ko=()):
    nc = bacc.Bacc("TRN2")

    xTd = nc.dram_tensor("xT", [D, S], F32R, kind="ExternalInput")
    wproj = nc.dram_tensor("wproj", [128, 8, 384], F32R, kind="ExternalInput")
    wout = nc.dram_tensor("wout", [128, 8, 128], F32R, kind="ExternalInput")
    pos3 = nc.dram_tensor("pos3", [3, S], I32, kind="ExternalInput")
    ropetab = nc.dram_tensor("ropetab", [96, 256], F32R, kind="ExternalInput")
    maug = nc.dram_tensor("maug", [96, HPC, 96], F32R, kind="ExternalInput")
    iota96 = nc.dram_tensor("iota96", [96, 1], F32, kind="ExternalInput")
    outT = nc.dram_tensor("outT", [128, S], F32, kind="ExternalOutput")
    dbg = {}
    if debug:
        for nm, shp in [("d_U", [96, S]), ("d_cosf", [128, S]),
                        ("d_sinf", [128, S]), ("d_qr", [128, S]),
                        ("d_kr", [128, S]), ("d_qaug", [96, HPC * S]),
                        ("d_vnat", [128, NKT * 130]),
                        ("d_call", [128, 8 * S])]:
            dbg[nm] = nc.dram_tensor(nm, shp, F32R, kind="ExternalOutput")
        for nm, shp in [("d_a0", [65, S]), ("d_a1", [65, S]),
                        ("d_agin", [128, S])]:
            dbg[nm] = nc.dram_tensor(nm, shp, F32, kind="ExternalOutput")

    with tile.TileContext(nc) as tc:
        with (
            tc.tile_pool(name="persist", bufs=1) as pp,
            tc.tile_pool(name="dram", bufs=1, space="DRAM") as dp,
        ):
            # ---- persistent tiles (live across phases) ----
            U = pp.tile([96, SP], F32R, name="U")
            qr = pp.tile([128, SP], F32R, name="qr")
            kr = pp.tile([128, SP], F32R, name="kr")
            qaug = pp.tile([96, HPC, SP], F32R, name="qaug")
            v_nat = pp.tile([128, NKT, 130], F32R, name="v_nat")
            ident = pp.tile([128, 128], F32R, name="ident")
            iot = pp.tile([96, 1], F32, name="iot")
            agin_s = pp.tile([128, S], F32R, name="agin_s")

            nc.sync.dma_start(iot[:], iota96[:])
            zcolF = pp.tile([128, 8], F32, name="zcolF")
            nc.gpsimd.memset(zcolF[:], 0.0)
            nc.vector.tensor_copy(U[:, S:SP], zcolF[0:96, 0:1])
            nc.vector.tensor_copy(qr[:, S:SP], zcolF[:, 0:1])
            nc.vector.tensor_copy(kr[:, S:SP], zcolF[:, 0:1])
            nc.vector.tensor_copy(
                qaug[:, :, S:SP].rearrange("p h x -> p (h x)"),
                zcolF[0:96, 0:2])
            attnh = [pp.tile([65, S], F32, name=f"attnh{h}")
                     for h in range(HPC)]
            zrecs = [pp.tile([1, S], F32, name=f"zrec{h}")
                     for h in range(HPC)]
            zbs = [pp.tile([64, S], F32, name=f"zb{h}") for h in range(HPC)]
            identF = pp.tile([128, 128], F32, name="identF")
            make_identity(nc, identF[:])
            nc.vector.tensor_copy(ident[:], identF[:])
            # ones columns of v_nat (cols 64 and 129 of each k-tile block)
            onesF = pp.tile([128, NKT, 2], F32, name="onesF")
            nc.gpsimd.memset(onesF[:], 1.0)
            ones_ap = v_nat[:].rearrange("p k (h x) -> p k h x", h=2)[:, :, :, 64:65]
            nc.vector.tensor_copy(ones_ap.rearrange("p k h x -> p k (h x)"),
                                  onesF[:])

            aexp_pool = tc.alloc_tile_pool(name="aexp_pool", bufs=5)
            # =============== phase A: U, rope tables, projections ===========
            with (
                tc.tile_pool(name="phA", bufs=1) as pa,
                tc.tile_pool(name="psA", bufs=1, space="PSUM") as psa,
            ):
                # --- one-hot U[axis*32+p, s] = (pos_axis[s] == p) ---
                # single cast-DMA: axis-major, each axis row replicated 32x
                posb = pa.tile([96, S], F32, name="posb")
                pos_rep = bass.AP(
                    tensor=pos3[:, :].tensor, offset=0,
                    ap=[[S, 3], [0, 32], [1, S]],
                )
                nc.gpsimd.dma_start(posb[:], pos_rep)
                nc.vector.tensor_scalar(
                    out=U[:, 0:S], in0=posb[:], scalar1=iot[:], scalar2=None,
                    op0=mybir.AluOpType.is_equal,
                )

                # --- cos/sin fields [128, S] via one-hot matmul ---
                rtab = pa.tile([96, 256], F32R, name="rtab")
                nc.sync.dma_start(rtab[:], ropetab[:])
                cosf = pa.tile([128, S], F32R, name="cosf")
                sinf = pa.tile([128, S], F32R, name="sinf")
                for ci, (q0, qw) in (() if "csaug" in ko
                                     else enumerate(CHUNKS)):
                    qe = qw + (qw % 2)
                    pcs = psa.tile([128, 512], F32, tag="mm", bufs=2,
                                   name=f"pcs{ci}")
                    nc.tensor.matmul(pcs[:, :qe], rtab[:, 0:128],
                                     U[:, q0 : q0 + qe], start=True, stop=True)
                    nc.scalar.copy(cosf[:, q0 : q0 + qw], pcs[:, :qw])
                    psn = psa.tile([128, 512], F32, tag="mm", bufs=2,
                                   name=f"psn{ci}")
                    nc.tensor.matmul(psn[:, :qe], rtab[:, 128:256],
                                     U[:, q0 : q0 + qe], start=True, stop=True)
                    nc.scalar.copy(sinf[:, q0 : q0 + qw], psn[:, :qw])

                # --- bias augmentation rows for q~ ---
                mg = pa.tile([96, HPC, 96], F32R, name="mg")
                nc.sync.dma_start(mg[:], maug[:])
                for h in (() if "csaug" in ko else range(HPC)):
                    for ci, (q0, qw) in enumerate(CHUNKS):
                        qe = qw + (qw % 2)
                        pag = psa.tile([128, 512], F32, tag="mm", bufs=2,
                                       name=f"pag{h}_{ci}")
                        nc.tensor.matmul(pag[0:96, :qe], mg[:, h, :],
                                         U[:, q0 : q0 + qe], start=True,
                                         stop=True)
                        nc.scalar.copy(qaug[:, h, q0 : q0 + qw],
                                       pag[0:96, :qw])

                # --- projections (per q-chunk), rope + v-transpose fused ---
                wp = pa.tile([128, 8, 384], F32R, name="wp")
                nc.scalar.dma_start(wp[:], wproj[:])
                with tc.tile_pool(name="pchunk", bufs=1) as pchk:
                    for ci, (q0, qw) in enumerate(CHUNKS):
                        qe = qw + (qw % 2)
                        xt = pchk.tile([128, 8, 512], F32R, tag="xt", bufs=2,
                                       name=f"xt{ci}")
                        for kt in range(8):
                            eng = nc.sync if kt % 2 == 0 else nc.scalar
                            eng.dma_start(
                                xt[:, kt, 0:qw],
                                xTd[128 * kt : 128 * kt + 128, q0 : q0 + qw])
                        if qe != qw:
                            nc.vector.tensor_copy(
                                xt[:, :, qw:qe].rearrange("p k x -> p (k x)"),
                                zcolF[:, 0:8])
                        outs = []
                        for mb, mbn in enumerate(["q2c", "k2c", "vTc"]):
                            ps = psa.tile([128, 512], F32, tag=f"pj{mb}",
                                          bufs=1, name=f"pj{mb}_{ci}")
                            for kt in range(8):
                                nc.tensor.matmul(
                                    ps[:, :qe],
                                    wp[:, kt, 128 * mb : 128 * mb + 128],
                                    xt[:, kt, 0:qe],
                                    start=(kt == 0), stop=(kt == 7),
                                )
                            dst = pchk.tile([128, 512], F32R, tag=mbn,
                                            bufs=2, name=f"{mbn}_{ci}")
                            if mb == 0:
                                nc.scalar.mul(dst[:, :qw], ps[:, :qw], SCALE)
                            else:
                                nc.vector.tensor_copy(dst[:, :qw], ps[:, :qw])
                            outs.append(dst)
                        q2c, k2c, vTc = outs
                        # rotate-half: rh[a] = -src[b], rh[b] = +src[a]
                        qrh2c = pchk.tile([128, 512], F32R, tag="qrh2c",
                                          bufs=2, name=f"qrh2c_{ci}")
                        krh2c = pchk.tile([128, 512], F32R, tag="krh2c",
                                          bufs=2, name=f"krh2c_{ci}")
                        for src, rh in ((q2c, qrh2c), (k2c, krh2c)):
                            for h in range(HPC):
                                a, b = 64 * h, 64 * h + 32
                                nc.gpsimd.tensor_scalar_mul(
                                    rh[a : a + 32, :qw],
                                    src[b : b + 32, :qw], -1.0)
                                nc.gpsimd.tensor_copy(
                                    rh[b : b + 32, :qw],
                                    src[a : a + 32, :qw])
                        # rope for this chunk
                        if "rope" in ko:
                            continue
                        tq = pchk.tile([128, 512], F32R, tag="tq", bufs=2,
                                       name=f"tq_{ci}")
                        nc.vector.tensor_mul(tq[:, :qw], qrh2c[:, :qw],
                                             sinf[:, q0 : q0 + qw])
                        nc.vector.tensor_mul(qr[:, q0 : q0 + qw], q2c[:, :qw],
                                             cosf[:, q0 : q0 + qw])
                        nc.vector.tensor_add(qr[:, q0 : q0 + qw],
                                             qr[:, q0 : q0 + qw], tq[:, :qw])
                        tk = pchk.tile([128, 512], F32R, tag="tk", bufs=2,
                                       name=f"tk_{ci}")
                        nc.gpsimd.tensor_mul(tk[:, :qw], krh2c[:, :qw],
                                             sinf[:, q0 : q0 + qw])
                        nc.gpsimd.tensor_mul(kr[:, q0 : q0 + qw], k2c[:, :qw],
                                             cosf[:, q0 : q0 + qw])
                        nc.gpsimd.tensor_add(kr[:, q0 : q0 + qw],
                                             kr[:, q0 : q0 + qw], tk[:, :qw])
                        # v transpose for k-tiles inside this chunk
                        for kt, (k0, kw) in enumerate(KTILES):
                            if not (q0 <= k0 < q0 + qw):
                                continue
                            off = k0 - q0
                            pvt = psa.tile([128, 512], F32R, tag="vt", bufs=1,
                                           name=f"pvt{kt}")
                            nc.tensor.transpose(pvt[0:kw, 0:128],
                                                vTc[:, off : off + kw],
                                                ident[:])
                            nc.vector.tensor_copy(
                                v_nat[0:kw, kt, :].rearrange(
                                    "p (h x) -> p h x", h=2)[:, :, 0:64],
                                pvt[0:kw, 0:128].rearrange(
                                    "p (h d) -> p h d", h=2),
                            )

            if debug:
                nc.sync.dma_start(dbg["d_U"][:], U[:, 0:S])
                nc.sync.dma_start(dbg["d_cosf"][:], cosf[:])
                nc.sync.dma_start(dbg["d_sinf"][:], sinf[:])
                nc.sync.dma_start(dbg["d_qr"][:], qr[:, 0:S])
                nc.sync.dma_start(dbg["d_kr"][:], kr[:, 0:S])
                nc.sync.dma_start(
                    dbg["d_qaug"][:].rearrange("p (h x) -> p h x", h=HPC),
                    qaug[:, :, 0:S])
                nc.sync.dma_start(
                    dbg["d_vnat"][:].rearrange("p (k x) -> p k x", k=NKT),
                    v_nat[:])

            if stop_after == "A":
                nc.sync.dma_start(outT[:], qr[:, 0:S].bitcast(F32))
            if stop_after is None or stop_after >= "B":
                # ======== phase B+C: attention, halved AG, out projection ==
                # asymmetric split: AG0 (chunks 0-2) overlaps chunk 3's
                # attention (the longest); AG1 is only 327 cols of tail
                HOFF = [0, 1536]
                HWID = [1536, S - 1536]  # 1536 + 327
                HCHUNKS = [[0, 1, 2], [3]]
                agin_d = [dp.tile([128, HWID[i]], F32R, name=f"agin{i}")
                          for i in range(2)]
                agout_d = [dp.tile([NC * 128, HWID[i]], F32R,
                                   addr_space="Shared", name=f"agout{i}")
                           for i in range(2)]
                with (
                    tc.tile_pool(name="phB", bufs=1) as pb,
                    tc.tile_pool(name="psB", bufs=1, space="PSUM") as psb,
                ):
                    ap = aexp_pool
                    wo = pb.tile([128, 8, 128], F32R, name="wo")
                    nc.sync.dma_start(wo[:], wout[:])
                    attn_all = pb.tile([128, 8, SP], F32R, name="attn_all")
                    nc.vector.tensor_copy(
                        attn_all[:, :, S:SP].rearrange("p k x -> p (k x)"),
                        zcolF[:, 0:8])
                    outTs = pb.tile([128, S], F32, name="outTs")

                    for half in range(2):
                        for ci in HCHUNKS[half]:
                            q0, qw = CHUNKS[ci]
                            qe = qw + (qw % 2)
                            qend = q0 + qe
                            kts = [kt for kt, (k0, kw) in enumerate(KTILES)
                                   if k0 <= q0 + qw - 1]
                            for h in range(HPC):
                                po = psb.tile([65, 512], F32, tag="pv",
                                              bufs=2, name=f"po{h}_{ci}")
                                for kt in kts:
                                    k0, kw = KTILES[kt]
                                    # diagonal trim: skip cols left of k0
                                    qlo = max(q0, min(k0, qend - 256))
                                    N = qend - qlo
                                    pss = psb.tile(
                                        [128, 512], F32, tag="sc", bufs=4,
                                        name=f"ps{h}_{ci}_{kt}")
                                    nc.tensor.matmul(
                                        pss[0:kw, :N],
                                        kr[64 * h : 64 * h + 64,
                                           k0 : k0 + kw],
                                        qr[64 * h : 64 * h + 64, qlo:qend],
                                        start=True, stop=False,
                                    )
                                    nc.tensor.matmul(
                                        pss[0:kw, :N],
                                        U[:, k0 : k0 + kw],
                                        qaug[:, h, qlo:qend],
                                        start=False, stop=True,
                                    )
                                    aexp = ap.tile([128, 512], F32R,
                                                   tag="ax",
                                                   name=f"ax{h}_{ci}_{kt}")
                                    nc.scalar.activation(
                                        aexp[0:kw, :N], pss[0:kw, :N],
                                        mybir.ActivationFunctionType.Exp,
                                    )
                                    if k0 + kw - 1 > qlo:
                                        W = min(q0 + qw, k0 + kw - 1) - qlo
                                        nc.gpsimd.affine_select(
                                            out=aexp[0:kw, 0:W],
                                            in_=aexp[0:kw, 0:W],
                                            compare_op=mybir.AluOpType.is_ge,
                                            fill=0.0, base=qlo - k0,
                                            channel_multiplier=-1,
                                            pattern=[[1, W]],
                                        )
                                    nc.tensor.matmul(
                                        po[:, qlo - q0 : qe],
                                        v_nat[0:kw, kt,
                                              65 * h : 65 * h + 65],
                                        aexp[0:kw, :N],
                                        start=(kt == kts[0]),
                                        stop=(kt == kts[-1]),
                                    )
                                nc.vector.tensor_copy(
                                    attnh[h][:, q0 : q0 + qw], po[:, :qw])

                        # normalize this half and start its all-gather
                        c0, cw = HOFF[half], HWID[half]
                        for h in range(HPC):
                            nc.vector.reciprocal(
                                zrecs[h][:, c0 : c0 + cw],
                                attnh[h][64:65, c0 : c0 + cw])
                            nc.gpsimd.partition_broadcast(
                                zbs[h][:, c0 : c0 + cw],
                                zrecs[h][:, c0 : c0 + cw])
                            nc.vector.tensor_mul(
                                agin_s[64 * h : 64 * h + 64, c0 : c0 + cw],
                                attnh[h][0:64, c0 : c0 + cw],
                                zbs[h][:, c0 : c0 + cw])
                        nc.sync.dma_start(agin_d[half][:],
                                          agin_s[:, c0 : c0 + cw])
                        if no_cc:
                            rep = bass.AP(
                                tensor=agin_d[half].tensor,
                                offset=agin_d[half].offset,
                                ap=[[0, 8], [cw, 128], [1, cw]],
                            )
                            nc.sync.dma_start(agout_d[half][:], rep)
                        else:
                            nc.gpsimd.collective_compute(
                                "AllGather",
                                mybir.AluOpType.bypass,
                                ins=[agin_d[half].opt()],
                                outs=[agout_d[half].opt()],
                                replica_groups=[list(range(NC))],
                            )
                        # load gathered half and run its out-projection
                        for g, eng in ((0, nc.sync), (1, nc.scalar)):
                            ag_rep = bass.AP(
                                tensor=agout_d[half].tensor,
                                offset=agout_d[half].offset + g * 4 * 128 * cw,
                                ap=[[cw, 128], [128 * cw, 4], [1, cw]],
                            )
                            eng.dma_start(
                                attn_all[:, 4 * g : 4 * g + 4, c0 : c0 + cw],
                                ag_rep)
                        for ci in HCHUNKS[half]:
                            q0, qw = CHUNKS[ci]
                            qe = qw + (qw % 2)
                            pso = psb.tile([128, 512], F32, tag="op",
                                           bufs=2, name=f"pso{ci}")
                            for kt in range(8):
                                nc.tensor.matmul(
                                    pso[:, :qe], wo[:, kt, :],
                                    attn_all[:, kt, q0 : q0 + qe],
                                    start=(kt == 0), stop=(kt == 7),
                                )
                            nc.vector.tensor_copy(outTs[:, q0 : q0 + qw],
                                                  pso[:, :qw])
                            nc.sync.dma_start(outT[:, q0 : q0 + qw],
                                              outTs[:, q0 : q0 + qw])

                    if debug:
                        nc.sync.dma_start(dbg["d_a0"][:], attnh[0][:])
                        nc.sync.dma_start(dbg["d_a1"][:], attnh[1][:])
                        nc.sync.dma_start(dbg["d_agin"][:], agin_s[:].bitcast(F32))
                        nc.sync.dma_start(
                            dbg["d_call"][:].rearrange("p (k x) -> p k x",
                                                       k=8),
                            attn_all[:, :, 0:S])
            if stop_after == "B":
                nc.sync.dma_start(outT[:], agin_s[:].bitcast(F32))
            aexp_pool.release()
    nc.compile()
    return nc


def _host_prep(x, Wqkv, Wout, bias_x, bias_y, bias_z, pos_xyz):
    """Shard/relayout inputs for the 8 cores. Pure relayout + constant tables."""
    xT = np.ascontiguousarray(x[0].T).astype(np.float32)  # [1024, S]
    pos3 = np.ascontiguousarray(pos_xyz[0].T).astype(np.int32)  # [3, S]

    # rope tables: lhsT [96, 128]; rows u=(axis,p), cols r (t = r % 32)
    invf = np.concatenate([
        BASE ** (-np.arange(0, 2 * PX, 2) / (2 * PX)),
        BASE ** (-np.arange(0, 2 * PY, 2) / (2 * PY)),
        BASE ** (-np.arange(0, 2 * PZ, 2) / (2 * PZ)),
    ]).astype(np.float64)  # [32]
    axis_of_t = np.array([0] * PX + [1] * PY + [2] * PZ)
    ropetab = np.zeros((96, 256), dtype=np.float32)
    p = np.arange(32)
    for r in range(128):
        t = r % 32
        ax = axis_of_t[t]
        p_eff = np.minimum(p, 7) if ax == 2 else p
        ang = p_eff * invf[t]
        ropetab[32 * ax : 32 * ax + 32, r] = np.cos(ang)
        ropetab[32 * ax : 32 * ax + 32, 128 + r] = np.sin(ang)

    iota96 = (np.arange(96) % 32).astype(np.float32).reshape(96, 1)

    # bias M matrices: M_ax[p_q, p_k] = bias_ax[clip(p_q - p_k, -c, c) + c, h]
    pq, pk = np.meshgrid(np.arange(32), np.arange(32), indexing="ij")
    dxy = np.clip(pq - pk, -MAX_DXY, MAX_DXY) + MAX_DXY
    dz = np.clip(pq - pk, -MAX_DZ, MAX_DZ) + MAX_DZ

    in_maps = []
    for c in range(NC):
        wblocks = []
        for blk in range(2):  # q-side then k-side
            wq = []
            for h in (HPC * c, HPC * c + 1):
                W = Wqkv[blk * D + HD * h : blk * D + HD * h + HD, :]  # [64,1024]
                wq.append(W[PERM])
            wblocks.append(np.concatenate(wq, axis=0))   # [128, 1024]
        wv = np.concatenate(
            [Wqkv[2 * D + HD * h : 2 * D + HD * h + HD, :]
             for h in (HPC * c, HPC * c + 1)], axis=0)
        wblocks.append(wv)
        # [384, 1024] -> lhsT layout [128(p within ktile), 8(ktile), 384(m)]
        wcat = np.concatenate(wblocks, axis=0)  # [384, 1024]
        wproj = np.ascontiguousarray(
            wcat.T.reshape(8, 128, 384).transpose(1, 0, 2)).astype(np.float32)

        woutT = Wout[128 * c : 128 * c + 128, :].T  # [1024, 128]
        wout_l = np.ascontiguousarray(
            woutT.reshape(8, 128, 128).transpose(1, 0, 2)).astype(np.float32)

        mg = np.zeros((96, HPC, 96), dtype=np.float32)
        for i, h in enumerate((HPC * c, HPC * c + 1)):
            mg[0:32, i, 0:32] = bias_x[dxy, h]
            mg[32:64, i, 32:64] = bias_y[dxy, h]
            mg[64:96, i, 64:96] = bias_z[dz, h]

        in_maps.append({
            "xT": xT, "wproj": wproj, "wout": wout_l, "pos3": pos3,
            "ropetab": ropetab, "maug": mg, "iota96": iota96,
        })
    return in_maps


_NC_CACHE = {}


def kernel(x, Wqkv, Wout, bias_x, bias_y, bias_z, pos_xyz, _bench=[None]):
    x = np.asarray(x, dtype=np.float32)
    Wqkv = np.asarray(Wqkv, dtype=np.float32)
    Wout = np.asarray(Wout, dtype=np.float32)
    bias_x = np.asarray(bias_x, dtype=np.float32)
    bias_y = np.asarray(bias_y, dtype=np.float32)
    bias_z = np.asarray(bias_z, dtype=np.float32)
    pos_xyz = np.asarray(pos_xyz, dtype=np.int32)

    in_maps = _host_prep(x, Wqkv, Wout, bias_x, bias_y, bias_z, pos_xyz)
    if "nc" not in _NC_CACHE:
        _NC_CACHE["nc"] = _build_nc()
    nc = _NC_CACHE["nc"]
    res = run_bass_kernel_spmd(nc, in_maps, core_ids=list(range(NC)))
    _bench[0] = res
    outT = np.concatenate([res.results[c]["outT"] for c in range(NC)], axis=0)
    return np.ascontiguousarray(outT.T).reshape(B, S, D).astype(np.float32)


# revision 45
# speedup vs baseline: 1.0959x; 1.0002x over previous
"""MultiHeadSelfAttention with 3D RoPE + 3D relative position bias on 8 trn2 cores.

Sharding: tensor-parallel over heads (2 heads per core). Each core:
  - projects q/k/v for its 2 heads (plus rotate-half projections for RoPE),
  - computes scores^T = k~.q~ with the 3D relative bias folded into the
    matmul contraction via one-hot position encodings,
  - causal-masked unnormalized-exp softmax (scores bounded, no max needed),
  - PV matmul with an appended ones-column producing softmax denominators,
  - AllGather of per-head attention outputs, then a row-slice of the final
    output projection.
Host does only sharding/relayout of inputs and the final gather/transpose.
"""
import numpy as np

import concourse.bass as bass
import concourse.bacc as bacc
import concourse.tile as tile
import concourse.mybir as mybir
from concourse.bass_utils import run_bass_kernel_spmd
from concourse.masks import make_identity

B, S, D, H, HD = 1, 1863, 1024, 16, 64
NC = 8
HPC = H // NC  # heads per core = 2
BASE = 10000.0
MAX_DXY, MAX_DZ = 30, 8
PX = PY = 10
PZ = 12
SCALE = HD ** -0.5

F32 = mybir.dt.float32
F32R = mybir.dt.float32r
I32 = mybir.dt.int32

# q-chunks (PSUM bank = 512 fp32) and k-tiles (128 partitions)
CHUNKS = [(c * 512, min(512, S - c * 512)) for c in range((S + 511) // 512)]
KTILES = [(kt * 128, min(128, S - kt * 128)) for kt in range((S + 127) // 128)]
NKT = len(KTILES)  # 15
SP = S + 1  # f32r matmuls need even moving-dim N; pad rhs tensors to 1864
PERM = np.concatenate([np.arange(0, HD, 2), np.arange(1, HD, 2)])  # even, odd


def _build_nc(debug=False, stop_after=None, no_cc=False, ko=()):
    nc = bacc.Bacc("TRN2")

    xTd = nc.dram_tensor("xT", [D, S], F32R, kind="ExternalInput")
    wproj = nc.dram_tensor("wproj", [128, 8, 384], F32R, kind="ExternalInput")
    wout = nc.dram_tensor("wout", [128, 8, 128], F32R, kind="ExternalInput")
    pos3 = nc.dram_tensor("pos3", [3, S], I32, kind="ExternalInput")
    ropetab = nc.dram_tensor("ropetab", [96, 256], F32R, kind="ExternalInput")
    maug = nc.dram_tensor("maug", [96, HPC, 96], F32R, kind="ExternalInput")
    iota96 = nc.dram_tensor("iota96", [96, 1], F32, kind="ExternalInput")
    outT = nc.dram_tensor("outT", [128, S], F32, kind="ExternalOutput")
    dbg = {}
    if debug:
        for nm, shp in [("d_U", [96, S]), ("d_cosf", [128, S]),
                        ("d_sinf", [128, S]), ("d_qr", [128, S]),
                        ("d_kr", [128, S]), ("d_qaug", [96, HPC * S]),
                        ("d_vnat", [128, NKT * 130]),
                        ("d_call", [128, 8 * S])]:
            dbg[nm] = nc.dram_tensor(nm, shp, F32R, kind="ExternalOutput")
        for nm, shp in [("d_a0", [65, S]), ("d_a1", [65, S]),
                        ("d_agin", [128, S])]:
            dbg[nm] = nc.dram_tensor(nm, shp, F32, kind="ExternalOutput")

    with tile.TileContext(nc) as tc:
        with (
            tc.tile_pool(name="persist", bufs=1) as pp,
            tc.tile_pool(name="dram", bufs=1, space="DRAM") as dp,
        ):
            # ---- persistent tiles (live across phases) ----
            U = pp.tile([96, SP], F32R, name="U")
            qr = pp.tile([128, SP], F32R, name="qr")
            kr = pp.tile([128, SP], F32R, name="kr")
            qaug = pp.tile([96, HPC, SP], F32R, name="qaug")
            v_nat = pp.tile([128, NKT, 130], F32R, name="v_nat")
            ident = pp.tile([128, 128], F32R, name="ident")
            iot = pp.tile([96, 1], F32, name="iot")
            agin_s = pp.tile([128, S], F32R, name="agin_s")

            nc.sync.dma_start(iot[:], iota96[:])
            zcolF = pp.tile([128, 8], F32, name="zcolF")
            nc.gpsimd.memset(zcolF[:], 0.0)
            nc.vector.tensor_copy(U[:, S:SP], zcolF[0:96, 0:1])
            nc.vector.tensor_copy(qr[:, S:SP], zcolF[:, 0:1])
            nc.vector.tensor_copy(kr[:, S:SP], zcolF[:, 0:1])
            nc.vector.tensor_copy(
                qaug[:, :, S:SP].rearrange("p h x -> p (h x)"),
                zcolF[0:96, 0:2])
            attnh = [pp.tile([65, S], F32, name=f"attnh{h}")
                     for h in range(HPC)]
            zrecs = [pp.tile([1, S], F32, name=f"zrec{h}")
                     for h in range(HPC)]
            zbs = [pp.tile([64, S], F32, name=f"zb{h}") for h in range(HPC)]
            identF = pp.tile([128, 128], F32, name="identF")
            make_identity(nc, identF[:])
            nc.vector.tensor_copy(ident[:], identF[:])
            # ones columns of v_nat (cols 64 and 129 of each k-tile block)
            onesF = pp.tile([128, NKT, 2], F32, name="onesF")
            nc.gpsimd.memset(onesF[:], 1.0)
            ones_ap = v_nat[:].rearrange("p k (h x) -> p k h x", h=2)[:, :, :, 64:65]
            nc.vector.tensor_copy(ones_ap.rearrange("p k h x -> p k (h x)"),
                                  onesF[:])

            aexp_pool = tc.alloc_tile_pool(name="aexp_pool", bufs=5)
            # =============== phase A: U, rope tables, projections ===========
            with (
                tc.tile_pool(name="phA", bufs=1) as pa,
                tc.tile_pool(name="psA", bufs=1, space="PSUM") as psa,
            ):
                # --- one-hot U[axis*32+p, s] = (pos_axis[s] == p) ---
                # single cast-DMA: axis-major, each axis row replicated 32x
                posb = pa.tile([96, S], F32, name="posb")
                pos_rep = bass.AP(
                    tensor=pos3[:, :].tensor, offset=0,
                    ap=[[S, 3], [0, 32], [1, S]],
                )
                nc.gpsimd.dma_start(posb[:], pos_rep)
                nc.vector.tensor_scalar(
                    out=U[:, 0:S], in0=posb[:], scalar1=iot[:], scalar2=None,
                    op0=mybir.AluOpType.is_equal,
                )

                # --- cos/sin fields [128, S] via one-hot matmul ---
                rtab = pa.tile([96, 256], F32R, name="rtab")
                nc.sync.dma_start(rtab[:], ropetab[:])
                cosf = pa.tile([128, S], F32R, name="cosf")
                sinf = pa.tile([128, S], F32R, name="sinf")
                for ci, (q0, qw) in (() if "csaug" in ko
                                     else enumerate(CHUNKS)):
                    qe = qw + (qw % 2)
                    pcs = psa.tile([128, 512], F32, tag="mm", bufs=2,
                                   name=f"pcs{ci}")
                    nc.tensor.matmul(pcs[:, :qe], rtab[:, 0:128],
                                     U[:, q0 : q0 + qe], start=True, stop=True)
                    nc.scalar.copy(cosf[:, q0 : q0 + qw], pcs[:, :qw])
                    psn = psa.tile([128, 512], F32, tag="mm", bufs=2,
                                   name=f"psn{ci}")
                    nc.tensor.matmul(psn[:, :qe], rtab[:, 128:256],
                                     U[:, q0 : q0 + qe], start=True, stop=True)
                    nc.scalar.copy(sinf[:, q0 : q0 + qw], psn[:, :qw])

                # --- bias augmentation rows for q~ ---
                mg = pa.tile([96, HPC, 96], F32R, name="mg")
                nc.sync.dma_start(mg[:], maug[:])
                for h in (() if "csaug" in ko else range(HPC)):
                    for ci, (q0, qw) in enumerate(CHUNKS):
                        qe = qw + (qw % 2)
                        pag = psa.tile([128, 512], F32, tag="mm", bufs=2,
                                       name=f"pag{h}_{ci}")
                        nc.tensor.matmul(pag[0:96, :qe], mg[:, h, :],
                                         U[:, q0 : q0 + qe], start=True,
                                         stop=True)
                        nc.scalar.copy(qaug[:, h, q0 : q0 + qw],
                                       pag[0:96, :qw])

                # --- projections (per q-chunk), rope + v-transpose fused ---
                wp = pa.tile([128, 8, 384], F32R, name="wp")
                nc.scalar.dma_start(wp[:], wproj[:])
                with tc.tile_pool(name="pchunk", bufs=1) as pchk:
                    for ci, (q0, qw) in enumerate(CHUNKS):
                        qe = qw + (qw % 2)
                        xt = pchk.tile([128, 8, 512], F32R, tag="xt", bufs=2,
                                       name=f"xt{ci}")
                        for kt in range(8):
                            eng = nc.sync if kt % 2 == 0 else nc.scalar
                            eng.dma_start(
                                xt[:, kt, 0:qw],
                                xTd[128 * kt : 128 * kt + 128, q0 : q0 + qw])
                        if qe != qw:
                            nc.vector.tensor_copy(
                                xt[:, :, qw:qe].rearrange("p k x -> p (k x)"),
                                zcolF[:, 0:8])
                        outs = []
                        for mb, mbn in enumerate(["q2c", "k2c", "vTc"]):
                            ps = psa.tile([128, 512], F32, tag=f"pj{mb}",
                                          bufs=1, name=f"pj{mb}_{ci}")
                            for kt in range(8):
                                nc.tensor.matmul(
                                    ps[:, :qe],
                                    wp[:, kt, 128 * mb : 128 * mb + 128],
                                    xt[:, kt, 0:qe],
                                    start=(kt == 0), stop=(kt == 7),
                                )
                            dst = pchk.tile([128, 512], F32R, tag=mbn,
                                            bufs=2, name=f"{mbn}_{ci}")
                            if mb == 0:
                                nc.scalar.mul(dst[:, :qw], ps[:, :qw], SCALE)
                            else:
                                nc.vector.tensor_copy(dst[:, :qw], ps[:, :qw])
                            outs.append(dst)
                        q2c, k2c, vTc = outs
                        # rotate-half: rh[a] = -src[b], rh[b] = +src[a]
                        qrh2c = pchk.tile([128, 512], F32R, tag="qrh2c",
                                          bufs=2, name=f"qrh2c_{ci}")
                        krh2c = pchk.tile([128, 512], F32R, tag="krh2c",
                                          bufs=2, name=f"krh2c_{ci}")
                        for src, rh in ((q2c, qrh2c), (k2c, krh2c)):
                            for h in range(HPC):
                                a, b = 64 * h, 64 * h + 32
                                nc.gpsimd.tensor_scalar_mul(
                                    rh[a : a + 32, :qw],
                                    src[b : b + 32, :qw], -1.0)
                                nc.gpsimd.tensor_copy(
                                    rh[b : b + 32, :qw],
                                    src[a : a + 32, :qw])
                        # rope for this chunk
                        if "rope" in ko:
                            continue
                        tq = pchk.tile([128, 512], F32R, tag="tq", bufs=2,
                                       name=f"tq_{ci}")
                        nc.vector.tensor_mul(tq[:, :qw], qrh2c[:, :qw],
                                             sinf[:, q0 : q0 + qw])
                        nc.vector.tensor_mul(qr[:, q0 : q0 + qw], q2c[:, :qw],
                                             cosf[:, q0 : q0 + qw])
                        nc.vector.tensor_add(qr[:, q0 : q0 + qw],
                                             qr[:, q0 : q0 + qw], tq[:, :qw])
                        tk = pchk.tile([128, 512], F32R, tag="tk", bufs=2,
                                       name=f"tk_{ci}")
                        nc.gpsimd.tensor_mul(tk[:, :qw], krh2c[:, :qw],
                                             sinf[:, q0 : q0 + qw])
                        nc.gpsimd.tensor_mul(kr[:, q0 : q0 + qw], k2c[:, :qw],
                                             cosf[:, q0 : q0 + qw])
                        nc.gpsimd.tensor_add(kr[:, q0 : q0 + qw],
                                             kr[:, q0 : q0 + qw], tk[:, :qw])
                        # v transpose for k-tiles inside this chunk
                        for kt, (k0, kw) in enumerate(KTILES):
                            if not (q0 <= k0 < q0 + qw):
                                continue
                            off = k0 - q0
                            pvt = psa.tile([128, 512], F32R, tag="vt", bufs=1,
                                           name=f"pvt{kt}")
                            nc.tensor.transpose(pvt[0:kw, 0:128],
                                                vTc[:, off : off + kw],
                                                ident[:])
                            nc.vector.tensor_copy(
                                v_nat[0:kw, kt, :].rearrange(
                                    "p (h x) -> p h x", h=2)[:, :, 0:64],
                                pvt[0:kw, 0:128].rearrange(
                                    "p (h d) -> p h d", h=2),
                            )

            if debug:
                nc.sync.dma_start(dbg["d_U"][:], U[:, 0:S])
                nc.sync.dma_start(dbg["d_cosf"][:], cosf[:])
                nc.sync.dma_start(dbg["d_sinf"][:], sinf[:])
                nc.sync.dma_start(dbg["d_qr"][:], qr[:, 0:S])
                nc.sync.dma_start(dbg["d_kr"][:], kr[:, 0:S])
                nc.sync.dma_start(
                    dbg["d_qaug"][:].rearrange("p (h x) -> p h x", h=HPC),
                    qaug[:, :, 0:S])
                nc.sync.dma_start(
                    dbg["d_vnat"][:].rearrange("p (k x) -> p k x", k=NKT),
                    v_nat[:])

            if stop_after == "A":
                nc.sync.dma_start(outT[:], qr[:, 0:S].bitcast(F32))
            if stop_after is None or stop_after >= "B":
                # ======== phase B+C: attention, halved AG, out projection ==
                # asymmetric split: AG0 (chunks 0-2) overlaps chunk 3's
                # attention (the longest); AG1 is only 327 cols of tail
                HOFF = [0, 1536]
                HWID = [1536, S - 1536]  # 1536 + 327
                HCHUNKS = [[0, 1, 2], [3]]
                agin_d = [dp.tile([128, HWID[i]], F32R, name=f"agin{i}")
                          for i in range(2)]
                agout_d = [dp.tile([NC * 128, HWID[i]], F32R,
                                   addr_space="Shared", name=f"agout{i}")
                           for i in range(2)]
                with (
                    tc.tile_pool(name="phB", bufs=1) as pb,
                    tc.tile_pool(name="psB", bufs=1, space="PSUM") as psb,
                ):
                    ap = aexp_pool
                    wo = pb.tile([128, 8, 128], F32R, name="wo")
                    nc.sync.dma_start(wo[:], wout[:])
                    attn_all = pb.tile([128, 8, SP], F32R, name="attn_all")
                    nc.vector.tensor_copy(
                        attn_all[:, :, S:SP].rearrange("p k x -> p (k x)"),
                        zcolF[:, 0:8])
                    outTs = pb.tile([128, S], F32, name="outTs")

                    for half in range(2):
                        for ci in HCHUNKS[half]:
                            q0, qw = CHUNKS[ci]
                            qe = qw + (qw % 2)
                            qend = q0 + qe
                            kts = [kt for kt, (k0, kw) in enumerate(KTILES)
                                   if k0 <= q0 + qw - 1]
                            for h in range(HPC):
                                po = psb.tile([65, 512], F32, tag="pv",
                                              bufs=2, name=f"po{h}_{ci}")
                                for kt in kts:
                                    k0, kw = KTILES[kt]
                                    # diagonal trim: skip cols left of k0
                                    qlo = max(q0, min(k0, qend - 256))
                                    N = qend - qlo
                                    pss = psb.tile(
                                        [128, 512], F32, tag="sc", bufs=5,
                                        name=f"ps{h}_{ci}_{kt}")
                                    nc.tensor.matmul(
                                        pss[0:kw, :N],
                                        kr[64 * h : 64 * h + 64,
                                           k0 : k0 + kw],
                                        qr[64 * h : 64 * h + 64, qlo:qend],
                                        start=True, stop=False,
                                    )
                                    nc.tensor.matmul(
                                        pss[0:kw, :N],
                                        U[:, k0 : k0 + kw],
                                        qaug[:, h, qlo:qend],
                                        start=False, stop=True,
                                    )
                                    aexp = ap.tile([128, 512], F32R,
                                                   tag="ax",
                                                   name=f"ax{h}_{ci}_{kt}")
                                    nc.scalar.activation(
                                        aexp[0:kw, :N], pss[0:kw, :N],
                                        mybir.ActivationFunctionType.Exp,
                                    )
                                    if k0 + kw - 1 > qlo:
                                        W = min(q0 + qw, k0 + kw - 1) - qlo
                                        nc.gpsimd.affine_select(
                                            out=aexp[0:kw, 0:W],
                                            in_=aexp[0:kw, 0:W],
                                            compare_op=mybir.AluOpType.is_ge,
                                            fill=0.0, base=qlo - k0,
                                            channel_multiplier=-1,
                                            pattern=[[1, W]],
                                        )
                                    nc.tensor.matmul(
                                        po[:, qlo - q0 : qe],
                                        v_nat[0:kw, kt,
                                              65 * h : 65 * h + 65],
                                        aexp[0:kw, :N],
                                        start=(kt == kts[0]),
                                        stop=(kt == kts[-1]),
                                    )
                                nc.vector.tensor_copy(
                                    attnh[h][:, q0 : q0 + qw], po[:, :qw])

                        # normalize this half and start its all-gather
                        c0, cw = HOFF[half], HWID[half]
                        for h in range(HPC):
                            nc.vector.reciprocal(
                                zrecs[h][:, c0 : c0 + cw],
                                attnh[h][64:65, c0 : c0 + cw])
                            nc.gpsimd.partition_broadcast(
                                zbs[h][:, c0 : c0 + cw],
                                zrecs[h][:, c0 : c0 + cw])
                            nc.vector.tensor_mul(
                                agin_s[64 * h : 64 * h + 64, c0 : c0 + cw],
                                attnh[h][0:64, c0 : c0 + cw],
                                zbs[h][:, c0 : c0 + cw])
                        nc.sync.dma_start(agin_d[half][:],
                                          agin_s[:, c0 : c0 + cw])
                        if no_cc:
                            rep = bass.AP(
                                tensor=agin_d[half].tensor,
                                offset=agin_d[half].offset,
                                ap=[[0, 8], [cw, 128], [1, cw]],
                            )
                            nc.sync.dma_start(agout_d[half][:], rep)
                        else:
                            nc.gpsimd.collective_compute(
                                "AllGather",
                                mybir.AluOpType.bypass,
                                ins=[agin_d[half].opt()],
                                outs=[agout_d[half].opt()],
                                replica_groups=[list(range(NC))],
                            )
                        # load gathered half and run its out-projection
                        for g, eng in ((0, nc.sync), (1, nc.scalar)):
                            ag_rep = bass.AP(
                                tensor=agout_d[half].tensor,
                                offset=agout_d[half].offset + g * 4 * 128 * cw,
                                ap=[[cw, 128], [128 * cw, 4], [1, cw]],
                            )
                            eng.dma_start(
                                attn_all[:, 4 * g : 4 * g + 4, c0 : c0 + cw],
                                ag_rep)
                        for ci in HCHUNKS[half]:
                            q0, qw = CHUNKS[ci]
                            qe = qw + (qw % 2)
                            pso = psb.tile([128, 512], F32, tag="op",
                                           bufs=1, name=f"pso{ci}")
                            for kt in range(8):
                                nc.tensor.matmul(
                                    pso[:, :qe], wo[:, kt, :],
                                    attn_all[:, kt, q0 : q0 + qe],
                                    start=(kt == 0), stop=(kt == 7),
                                )
                            nc.vector.tensor_copy(outTs[:, q0 : q0 + qw],
                                                  pso[:, :qw])
                            nc.sync.dma_start(outT[:, q0 : q0 + qw],
                                              outTs[:, q0 : q0 + qw])

                    if debug:
                        nc.sync.dma_start(dbg["d_a0"][:], attnh[0][:])
                        nc.sync.dma_start(dbg["d_a1"][:], attnh[1][:])
                        nc.sync.dma_start(dbg["d_agin"][:], agin_s[:].bitcast(F32))
                        nc.sync.dma_start(
                            dbg["d_call"][:].rearrange("p (k x) -> p k x",
                                                       k=8),
                            attn_all[:, :, 0:S])
            if stop_after == "B":
                nc.sync.dma_start(outT[:], agin_s[:].bitcast(F32))
            aexp_pool.release()
    nc.compile()
    return nc


def _host_prep(x, Wqkv, Wout, bias_x, bias_y, bias_z, pos_xyz):
    """Shard/relayout inputs for the 8 cores. Pure relayout + constant tables."""
    xT = np.ascontiguousarray(x[0].T).astype(np.float32)  # [1024, S]
    pos3 = np.ascontiguousarray(pos_xyz[0].T).astype(np.int32)  # [3, S]

    # rope tables: lhsT [96, 128]; rows u=(axis,p), cols r (t = r % 32)
    invf = np.concatenate([
        BASE ** (-np.arange(0, 2 * PX, 2) / (2 * PX)),
        BASE ** (-np.arange(0, 2 * PY, 2) / (2 * PY)),
        BASE ** (-np.arange(0, 2 * PZ, 2) / (2 * PZ)),
    ]).astype(np.float64)  # [32]
    axis_of_t = np.array([0] * PX + [1] * PY + [2] * PZ)
    ropetab = np.zeros((96, 256), dtype=np.float32)
    p = np.arange(32)
    for r in range(128):
        t = r % 32
        ax = axis_of_t[t]
        p_eff = np.minimum(p, 7) if ax == 2 else p
        ang = p_eff * invf[t]
        ropetab[32 * ax : 32 * ax + 32, r] = np.cos(ang)
        ropetab[32 * ax : 32 * ax + 32, 128 + r] = np.sin(ang)

    iota96 = (np.arange(96) % 32).astype(np.float32).reshape(96, 1)

    # bias M matrices: M_ax[p_q, p_k] = bias_ax[clip(p_q - p_k, -c, c) + c, h]
    pq, pk = np.meshgrid(np.arange(32), np.arange(32), indexing="ij")
    dxy = np.clip(pq - pk, -MAX_DXY, MAX_DXY) + MAX_DXY
    dz = np.clip(pq - pk, -MAX_DZ, MAX_DZ) + MAX_DZ

    in_maps = []
    for c in range(NC):
        wblocks = []
        for blk in range(2):  # q-side then k-side
            wq = []
            for h in (HPC * c, HPC * c + 1):
                W = Wqkv[blk * D + HD * h : blk * D + HD * h + HD, :]  # [64,1024]
                wq.append(W[PERM])
            wblocks.append(np.concatenate(wq, axis=0))   # [128, 1024]
        wv = np.concatenate(
            [Wqkv[2 * D + HD * h : 2 * D + HD * h + HD, :]
             for h in (HPC * c, HPC * c + 1)], axis=0)
        wblocks.append(wv)
        # [384, 1024] -> lhsT layout [128(p within ktile), 8(ktile), 384(m)]
        wcat = np.concatenate(wblocks, axis=0)  # [384, 1024]
        wproj = np.ascontiguousarray(
            wcat.T.reshape(8, 128, 384).transpose(1, 0, 2)).astype(np.float32)

        woutT = Wout[128 * c : 128 * c + 128, :].T  # [1024, 128]
        wout_l = np.ascontiguousarray(
            woutT.reshape(8, 128, 128).transpose(1, 0, 2)).astype(np.float32)

        mg = np.zeros((96, HPC, 96), dtype=np.float32)
        for i, h in enumerate((HPC * c, HPC * c + 1)):
            mg[0:32, i, 0:32] = bias_x[dxy, h]
            mg[32:64, i, 32:64] = bias_y[dxy, h]
            mg[64:96, i, 64:96] = bias_z[dz, h]

        in_maps.append({
            "xT": xT, "wproj": wproj, "wout": wout_l, "pos3": pos3,
            "ropetab": ropetab, "maug": mg, "iota96": iota96,
        })
    return in_maps


_NC_CACHE = {}


def kernel(x, Wqkv, Wout, bias_x, bias_y, bias_z, pos_xyz, _bench=[None]):
    x = np.asarray(x, dtype=np.float32)
    Wqkv = np.asarray(Wqkv, dtype=np.float32)
    Wout = np.asarray(Wout, dtype=np.float32)
    bias_x = np.asarray(bias_x, dtype=np.float32)
    bias_y = np.asarray(bias_y, dtype=np.float32)
    bias_z = np.asarray(bias_z, dtype=np.float32)
    pos_xyz = np.asarray(pos_xyz, dtype=np.int32)

    in_maps = _host_prep(x, Wqkv, Wout, bias_x, bias_y, bias_z, pos_xyz)
    if "nc" not in _NC_CACHE:
        _NC_CACHE["nc"] = _build_nc()
    nc = _NC_CACHE["nc"]
    res = run_bass_kernel_spmd(nc, in_maps, core_ids=list(range(NC)))
    _bench[0] = res
    outT = np.concatenate([res.results[c]["outT"] for c in range(NC)], axis=0)
    return np.ascontiguousarray(outT.T).reshape(B, S, D).astype(np.float32)
